# revision 10
# baseline (speedup 1.0000x reference)
"""Trainium2 Bass kernel for nn_BigFanoutModel (100 tiny fanout matmuls + sum).

Math: out[k] = sum_{n,d} x[0,d] * matrices[n,d,k] == x @ (sum_n matrices[n]).
Shapes: x (1,4) f32, matrices (100,4,4) f32 -> out (4,) f32.

Total input is 6.4KB, so the problem is pure latency. Per the sharding hint
("too small to shard meaningfully"), the full inputs are replicated on all 8
cores; every core computes the full output with a minimal instruction chain
and core 0's result is returned. No collectives.

Per-core dataflow (engines: SP=sync DMA, ACT=scalar DMA, DVE=vector, PE):
  SP   A_sb[100,16] <- matrices, contiguous (100 rows x 64B)
  ACT  x_sb[1,4]    <- x                  (parallel HWDGE queue)
  DVE  ones[100,1]  <- memset 1.0
  PE   U[1,16]      <- ones.T @ A_sb      (contracts n=100 in one matmul)
  DVE  W[1,16]      <- U * x              (x broadcast along k via stride-0 AP)
  DVE  res[1,4]     <- sum over d of W    (strided view, reduce X)
  SP   out[4]       <- res, then wait for the write receipt

Implementation notes:
- Raw Bass (no Tile): the whole kernel is ~9 instructions; Tile's scheduler
  and its kernel-tail barrier only add overhead at this size.
- "Lean" Bass construction: the const-AP memsets and the init-time
  all-engine barrier emitted by Bass.__init__ are suppressed (nothing here
  uses the const pool, and the NEFF's runtime prologue already synchronizes
  the engines). No Block() wrapper -> no exit barrier.
- The DVE mul->reduce pair carries an explicit same-engine semaphore wait:
  DVE pipelines back-to-back instructions, so the reduce would otherwise
  read w_sb before the multiply's writes land (confirmed by the CoreSim
  race detector and by a wrong result on hardware).
- fp32 matmul runs as a LOW/HIGH dual pass on the PE; keeping the moving
  free dim at N=16 makes each pass ~185ns (vs ~850ns at N=400).
- Measured on trn2 (NTFF profile, first-to-last instruction): ~18.0-18.9us
  total, of which ~14us is the runtime-injected NEFF prologue/epilogue
  (engine start + sem-file reset, identical for any kernel here) and ~4us
  is this kernel's body (dominated by the two HBM round trips).
"""

import numpy as np

import concourse.bass as bass
import concourse.mybir as mybir
from concourse.bass_utils import run_bass_kernel_spmd

N_CORES = 8

_NC_CACHE = None


def _make_bass_lean():
    """Bass() without the const-AP memsets and init all-engine barrier."""
    orig_barrier = bass.Bass.all_engine_barrier
    orig_memset = bass.BassGpSimd.memset
    bass.Bass.all_engine_barrier = lambda self, **k: None
    bass.BassGpSimd.memset = lambda self, ap, c: None
    try:
        nc = bass.Bass(monotonic_sem_count=0)
    finally:
        bass.Bass.all_engine_barrier = orig_barrier
        bass.BassGpSimd.memset = orig_memset
    return nc


def _build_nc():
    nc = _make_bass_lean()
    x = nc.dram_tensor("x", [1, 4], mybir.dt.float32, kind="ExternalInput")
    m = nc.dram_tensor("matrices", [100, 4, 4], mybir.dt.float32, kind="ExternalInput")
    o = nc.dram_tensor("out", [4], mybir.dt.float32, kind="ExternalOutput")
    with (
        nc.semaphore("semA") as semA,
        nc.semaphore("semX") as semX,
        nc.semaphore("semO") as semO,
        nc.semaphore("c") as c,
        nc.sbuf_tensor("A_sb", [100, 16], mybir.dt.float32) as A_sb,
        nc.sbuf_tensor("ones_sb", [100, 1], mybir.dt.float32) as ones_sb,
        nc.sbuf_tensor("x_sb", [1, 4], mybir.dt.float32) as x_sb,
        nc.sbuf_tensor("w_sb", [1, 16], mybir.dt.float32) as w_sb,
        nc.sbuf_tensor("res_sb", [1, 4], mybir.dt.float32) as res_sb,
        nc.psum_tensor("u_ps", [1, 16], mybir.dt.float32) as u_ps,
    ):
        # SP: matrices (the long-pole transfer); ACT: x in parallel.
        nc.sync.dma_start(
            bass.AP(A_sb, 0, [[16, 100], [1, 16]]),
            bass.AP(m, 0, [[16, 100], [1, 16]]),
        ).then_inc(semA, 16)
        nc.scalar.dma_start(
            bass.AP(x_sb, 0, [[4, 1], [1, 4]]),
            bass.AP(x, 0, [[4, 1], [1, 4]]),
        ).then_inc(semX, 16)

        # DVE: ones vector for the n-contraction.
        nc.vector.memset(bass.AP(ones_sb, 0, [[1, 100], [1, 1]]), 1.0).then_inc(c, 1)

        # PE: U[1,16] = ones.T @ A  == sum_n matrices[n], flattened (d,k).
        nc.tensor.wait_ge(c, 1)
        nc.tensor.wait_ge(semA, 16)
        nc.tensor.matmul(
            bass.AP(u_ps, 0, [[16, 1], [1, 16]]),
            bass.AP(ones_sb, 0, [[1, 100], [1, 1]]),
            bass.AP(A_sb, 0, [[16, 100], [1, 16]]),
        ).then_inc(c, 1)

        # DVE: W[d,k] = U[d,k] * x[d]; then res[k] = sum_d W[d,k].
        # semX first: x's receipt lands ~1us before the matmul finishes, so
        # this wait clears while PE is still busy; waiting on c last means
        # the multiply issues immediately after the matmul's increment.
        nc.vector.wait_ge(semX, 16)
        nc.vector.wait_ge(c, 2)
        nc.vector.tensor_mul(
            bass.AP(w_sb, 0, [[16, 1], [4, 4], [1, 4]]),
            bass.AP(u_ps, 0, [[16, 1], [4, 4], [1, 4]]),
            bass.AP(x_sb, 0, [[4, 1], [1, 4], [0, 4]]),
        ).then_inc(c, 1)
        nc.vector.wait_ge(c, 3)  # same-engine pipeline hazard on w_sb
        nc.vector.reduce_sum(
            out=bass.AP(res_sb, 0, [[4, 1], [1, 4]]),
            in_=bass.AP(w_sb, 0, [[16, 1], [1, 4], [4, 4]]),
            axis=mybir.AxisListType.X,
        ).then_inc(c, 1)

        # SP: out, with an explicit completion wait. (A fire-and-forget
        # variant saves ~1us but races the runtime's end-of-NEFF semaphore
        # reset; an NRT_EXEC_UNIT_UNRECOVERABLE was observed under repeated
        # executions without this wait, so keep it.)
        nc.sync.wait_ge(c, 4)
        # single_packet: 16B on one SDMA engine -> one completion stream,
        # no waiting on 15 dataless engines' semaphore increments. (Hurts
        # the big load, helps the tiny store.)
        nc.sync.dma_start(
            bass.AP(o, 0, [[1, 4]]),
            bass.AP(res_sb, 0, [[4, 1], [1, 4]]),
            single_packet=True,
        ).then_inc(semO, 16)
        nc.sync.wait_ge(semO, 16)
    return nc


def _get_nc():
    global _NC_CACHE
    if _NC_CACHE is None:
        _NC_CACHE = _build_nc()
    return _NC_CACHE


def _run(x, matrices, **kwargs):
    """Uncached path (kept for test harnesses that want BassKernelResults)."""
    nc = _get_nc()
    in_map = {
        "x": np.ascontiguousarray(x, dtype=np.float32),
        "matrices": np.ascontiguousarray(matrices, dtype=np.float32),
    }
    in_maps = [in_map for _ in range(N_CORES)]
    return run_bass_kernel_spmd(nc, in_maps, list(range(N_CORES)), **kwargs)


def kernel(x, matrices):
    # Fresh dispatch per call (the ecosystem-default run_bass_kernel_spmd
    # path). Each call executes the NEFF as a first execution, which has a
    # ~8us faster runtime prologue than re-executing a cached executable
    # (re-execution repeats the engine-state TENSOR_LOAD round). The
    # compiled NEFF itself comes from the on-disk neuron compile cache, so
    # per-call overhead is only the PJRT trace+load (~0.7s wall).
    res = _run(x, matrices)
    return np.asarray(res.results[0]["out"], dtype=np.float32).reshape(4)


# revision 11
# speedup vs baseline: 1.0033x; 1.0033x over previous
"""Trainium2 Bass kernel for nn_BigFanoutModel (100 tiny fanout matmuls + sum).

Math: out[k] = sum_{n,d} x[0,d] * matrices[n,d,k] == x @ (sum_n matrices[n]).
Shapes: x (1,4) f32, matrices (100,4,4) f32 -> out (4,) f32.

Total input is 6.4KB, so the problem is pure latency. Per the sharding hint
("too small to shard meaningfully"), the full inputs are replicated on all 8
cores; every core computes the full output with a minimal instruction chain
and core 0's result is returned. No collectives.

Per-core dataflow (engines: SP=sync DMA, ACT=scalar DMA, DVE=vector, PE):
  SP   A_sb[100,16] <- matrices, contiguous (100 rows x 64B)
  ACT  x_sb[1,4]    <- x                  (parallel HWDGE queue)
  DVE  ones[100,1]  <- memset 1.0
  PE   U[1,16]      <- ones.T @ A_sb      (contracts n=100 in one matmul)
  DVE  W[1,16]      <- U * x              (x broadcast along k via stride-0 AP)
  DVE  res[1,4]     <- sum over d of W    (strided view, reduce X)
  SP   out[4]       <- res, then wait for the write receipt

Implementation notes:
- Raw Bass (no Tile): the whole kernel is ~9 instructions; Tile's scheduler
  and its kernel-tail barrier only add overhead at this size.
- "Lean" Bass construction: the const-AP memsets and the init-time
  all-engine barrier emitted by Bass.__init__ are suppressed (nothing here
  uses the const pool, and the NEFF's runtime prologue already synchronizes
  the engines). No Block() wrapper -> no exit barrier.
- The DVE mul->reduce pair carries an explicit same-engine semaphore wait:
  DVE pipelines back-to-back instructions, so the reduce would otherwise
  read w_sb before the multiply's writes land (confirmed by the CoreSim
  race detector and by a wrong result on hardware).
- fp32 matmul runs as a LOW/HIGH dual pass on the PE; keeping the moving
  free dim at N=16 makes each pass ~185ns (vs ~850ns at N=400).
- Measured on trn2 (NTFF profile, first-to-last instruction): ~18.0-18.9us
  total, of which ~14us is the runtime-injected NEFF prologue/epilogue
  (engine start + sem-file reset, identical for any kernel here) and ~4us
  is this kernel's body (dominated by the two HBM round trips).
"""

import numpy as np

import concourse.bass as bass
import concourse.mybir as mybir
from concourse.bass_utils import run_bass_kernel_spmd

N_CORES = 8

_NC_CACHE = None


def _make_bass_lean():
    """Bass() without the const-AP memsets and init all-engine barrier."""
    orig_barrier = bass.Bass.all_engine_barrier
    orig_memset = bass.BassGpSimd.memset
    bass.Bass.all_engine_barrier = lambda self, **k: None
    bass.BassGpSimd.memset = lambda self, ap, c: None
    try:
        nc = bass.Bass(monotonic_sem_count=0)
    finally:
        bass.Bass.all_engine_barrier = orig_barrier
        bass.BassGpSimd.memset = orig_memset
    return nc


def _build_nc():
    nc = _make_bass_lean()
    x = nc.dram_tensor("x", [1, 4], mybir.dt.float32, kind="ExternalInput")
    m = nc.dram_tensor("matrices", [100, 4, 4], mybir.dt.float32, kind="ExternalInput")
    o = nc.dram_tensor("out", [4], mybir.dt.float32, kind="ExternalOutput")
    with (
        nc.semaphore("semA") as semA,
        nc.semaphore("semX") as semX,
        nc.semaphore("semO") as semO,
        nc.semaphore("c") as c,
        nc.sbuf_tensor("A_sb", [100, 16], mybir.dt.float32) as A_sb,
        nc.sbuf_tensor("ones_sb", [100, 1], mybir.dt.float32) as ones_sb,
        nc.sbuf_tensor("x_sb", [1, 4], mybir.dt.float32) as x_sb,
        nc.sbuf_tensor("w_sb", [1, 16], mybir.dt.float32) as w_sb,
        nc.sbuf_tensor("res_sb", [1, 4], mybir.dt.float32) as res_sb,
        nc.psum_tensor("u_ps", [1, 16], mybir.dt.float32) as u_ps,
    ):
        # SP: matrices (the long-pole transfer); ACT: x in parallel.
        nc.sync.dma_start(
            bass.AP(A_sb, 0, [[16, 100], [1, 16]]),
            bass.AP(m, 0, [[16, 100], [1, 16]]),
        ).then_inc(semA, 16)
        nc.scalar.dma_start(
            bass.AP(x_sb, 0, [[4, 1], [1, 4]]),
            bass.AP(x, 0, [[4, 1], [1, 4]]),
        ).then_inc(semX, 16)

        # DVE: ones vector for the n-contraction.
        nc.vector.memset(bass.AP(ones_sb, 0, [[1, 100], [1, 1]]), 1.0).then_inc(c, 1)

        # PE: U[1,16] = ones.T @ A  == sum_n matrices[n], flattened (d,k).
        nc.tensor.wait_ge(c, 1)
        nc.tensor.wait_ge(semA, 16)
        nc.tensor.matmul(
            bass.AP(u_ps, 0, [[16, 1], [1, 16]]),
            bass.AP(ones_sb, 0, [[1, 100], [1, 1]]),
            bass.AP(A_sb, 0, [[16, 100], [1, 16]]),
        ).then_inc(c, 1)

        # DVE: W[d,k] = U[d,k] * x[d]; then res[k] = sum_d W[d,k].
        # semX first: x's receipt lands ~1us before the matmul finishes, so
        # this wait clears while PE is still busy; waiting on c last means
        # the multiply issues immediately after the matmul's increment.
        nc.vector.wait_ge(semX, 16)
        nc.vector.wait_ge(c, 2)
        nc.vector.tensor_mul(
            bass.AP(w_sb, 0, [[16, 1], [4, 4], [1, 4]]),
            bass.AP(u_ps, 0, [[16, 1], [4, 4], [1, 4]]),
            bass.AP(x_sb, 0, [[4, 1], [1, 4], [0, 4]]),
        ).then_inc(c, 1)
        nc.vector.wait_ge(c, 3)  # same-engine pipeline hazard on w_sb
        nc.vector.reduce_sum(
            out=bass.AP(res_sb, 0, [[4, 1], [1, 4]]),
            in_=bass.AP(w_sb, 0, [[16, 1], [1, 4], [4, 4]]),
            axis=mybir.AxisListType.X,
        ).then_inc(c, 1)

        # SP: out, with an explicit completion wait. (A fire-and-forget
        # variant saves ~1us but races the runtime's end-of-NEFF semaphore
        # reset; an NRT_EXEC_UNIT_UNRECOVERABLE was observed under repeated
        # executions without this wait, so keep it.)
        nc.sync.wait_ge(c, 4)
        nc.sync.dma_start(
            bass.AP(o, 0, [[1, 4]]),
            bass.AP(res_sb, 0, [[4, 1], [1, 4]]),
        ).then_inc(semO, 16)
        nc.sync.wait_ge(semO, 16)
    return nc


def _get_nc():
    global _NC_CACHE
    if _NC_CACHE is None:
        _NC_CACHE = _build_nc()
    return _NC_CACHE


def _run(x, matrices, **kwargs):
    """Uncached path (kept for test harnesses that want BassKernelResults)."""
    nc = _get_nc()
    in_map = {
        "x": np.ascontiguousarray(x, dtype=np.float32),
        "matrices": np.ascontiguousarray(matrices, dtype=np.float32),
    }
    in_maps = [in_map for _ in range(N_CORES)]
    return run_bass_kernel_spmd(nc, in_maps, list(range(N_CORES)), **kwargs)


def kernel(x, matrices):
    # Fresh dispatch per call (the ecosystem-default run_bass_kernel_spmd
    # path). Each call executes the NEFF as a first execution, which has a
    # ~8us faster runtime prologue than re-executing a cached executable
    # (re-execution repeats the engine-state TENSOR_LOAD round). The
    # compiled NEFF itself comes from the on-disk neuron compile cache, so
    # per-call overhead is only the PJRT trace+load (~0.7s wall).
    res = _run(x, matrices)
    return np.asarray(res.results[0]["out"], dtype=np.float32).reshape(4)


# revision 13
# speedup vs baseline: 1.0041x; 1.0008x over previous
"""Trainium2 Bass kernel for nn_BigFanoutModel (100 tiny fanout matmuls + sum).

Math: out[k] = sum_{n,d} x[0,d] * matrices[n,d,k] == x @ (sum_n matrices[n]).
Shapes: x (1,4) f32, matrices (100,4,4) f32 -> out (4,) f32.

Total input is 6.4KB, so the problem is pure latency. Per the sharding hint
("too small to shard meaningfully"), the full inputs are replicated on all 8
cores; every core computes the full output with a minimal instruction chain
and core 0's result is returned. No collectives.

Per-core dataflow (engines: SP=sync DMA, ACT=scalar DMA, DVE=vector, PE):
  SP   A_sb[100,16] <- matrices, contiguous (100 rows x 64B)
  ACT  x_sb[1,4]    <- x                  (parallel HWDGE queue)
  DVE  ones[100,1]  <- memset 1.0
  PE   U[1,16]      <- ones.T @ A_sb      (contracts n=100 in one matmul)
  DVE  W[1,16]      <- U * x              (x broadcast along k via stride-0 AP)
  DVE  res[1,4]     <- sum over d of W    (strided view, reduce X)
  SP   out[4]       <- res, then wait for the write receipt

Implementation notes:
- Raw Bass (no Tile): the whole kernel is ~9 instructions; Tile's scheduler
  and its kernel-tail barrier only add overhead at this size.
- "Lean" Bass construction: the const-AP memsets and the init-time
  all-engine barrier emitted by Bass.__init__ are suppressed (nothing here
  uses the const pool, and the NEFF's runtime prologue already synchronizes
  the engines). No Block() wrapper -> no exit barrier.
- The DVE mul->reduce pair carries an explicit same-engine semaphore wait:
  DVE pipelines back-to-back instructions, so the reduce would otherwise
  read w_sb before the multiply's writes land (confirmed by the CoreSim
  race detector and by a wrong result on hardware).
- fp32 matmul runs as a LOW/HIGH dual pass on the PE; keeping the moving
  free dim at N=16 makes each pass ~185ns (vs ~850ns at N=400).
- Measured on trn2 (NTFF profile, first-to-last instruction): ~18.0-18.9us
  total, of which ~14us is the runtime-injected NEFF prologue/epilogue
  (engine start + sem-file reset, identical for any kernel here) and ~4us
  is this kernel's body (dominated by the two HBM round trips).
"""

import numpy as np

import concourse.bass as bass
import concourse.mybir as mybir
from concourse.bass_utils import run_bass_kernel_spmd

N_CORES = 8

_NC_CACHE = None


def _make_bass_lean():
    """Bass() without the const-AP memsets and init all-engine barrier."""
    orig_barrier = bass.Bass.all_engine_barrier
    orig_memset = bass.BassGpSimd.memset
    bass.Bass.all_engine_barrier = lambda self, **k: None
    bass.BassGpSimd.memset = lambda self, ap, c: None
    try:
        nc = bass.Bass(monotonic_sem_count=0)
    finally:
        bass.Bass.all_engine_barrier = orig_barrier
        bass.BassGpSimd.memset = orig_memset
    return nc


def _build_nc():
    nc = _make_bass_lean()
    x = nc.dram_tensor("x", [1, 4], mybir.dt.float32, kind="ExternalInput")
    m = nc.dram_tensor("matrices", [100, 4, 4], mybir.dt.float32, kind="ExternalInput")
    o = nc.dram_tensor("out", [4], mybir.dt.float32, kind="ExternalOutput")
    with (
        nc.semaphore("semA") as semA,
        nc.semaphore("semX") as semX,
        nc.semaphore("semO") as semO,
        nc.semaphore("c") as c,
        nc.sbuf_tensor("A_sb", [100, 16], mybir.dt.float32) as A_sb,
        nc.sbuf_tensor("ones_sb", [100, 1], mybir.dt.float32) as ones_sb,
        nc.sbuf_tensor("x_sb", [1, 4], mybir.dt.float32) as x_sb,
        nc.sbuf_tensor("w_sb", [1, 16], mybir.dt.float32) as w_sb,
        nc.sbuf_tensor("res_sb", [1, 4], mybir.dt.float32) as res_sb,
        nc.psum_tensor("u_ps", [1, 16], mybir.dt.float32) as u_ps,
    ):
        # SP: matrices (the long-pole transfer); ACT: x in parallel.
        nc.sync.dma_start(
            bass.AP(A_sb, 0, [[16, 100], [1, 16]]),
            bass.AP(m, 0, [[16, 100], [1, 16]]),
        ).then_inc(semA, 16)
        nc.scalar.dma_start(
            bass.AP(x_sb, 0, [[4, 1], [1, 4]]),
            bass.AP(x, 0, [[4, 1], [1, 4]]),
        ).then_inc(semX, 16)

        # DVE: ones vector for the n-contraction.
        nc.vector.memset(bass.AP(ones_sb, 0, [[1, 100], [1, 1]]), 1.0).then_inc(c, 1)

        # PE: U[1,16] = ones.T @ A  == sum_n matrices[n], flattened (d,k).
        nc.tensor.wait_ge(c, 1)
        nc.tensor.wait_ge(semA, 16)
        nc.tensor.matmul(
            bass.AP(u_ps, 0, [[16, 1], [1, 16]]),
            bass.AP(ones_sb, 0, [[1, 100], [1, 1]]),
            bass.AP(A_sb, 0, [[16, 100], [1, 16]]),
        ).then_inc(c, 1)

        # DVE: W[d,k] = U[d,k] * x[d]; then res[k] = sum_d W[d,k].
        # semX first: x's receipt lands ~1us before the matmul finishes, so
        # this wait clears while PE is still busy; waiting on c last means
        # the multiply issues immediately after the matmul's increment.
        nc.vector.wait_ge(semX, 16)
        nc.vector.wait_ge(c, 2)
        nc.vector.tensor_mul(
            bass.AP(w_sb, 0, [[16, 1], [4, 4], [1, 4]]),
            bass.AP(u_ps, 0, [[16, 1], [4, 4], [1, 4]]),
            bass.AP(x_sb, 0, [[4, 1], [1, 4], [0, 4]]),
        ).then_inc(c, 1)
        nc.vector.wait_ge(c, 3)  # same-engine pipeline hazard on w_sb
        nc.vector.reduce_sum(
            out=bass.AP(res_sb, 0, [[4, 1], [1, 4]]),
            in_=bass.AP(w_sb, 0, [[16, 1], [1, 4], [4, 4]]),
            axis=mybir.AxisListType.X,
        ).then_inc(c, 1)

        # SP: out, with an explicit completion wait. (A fire-and-forget
        # variant saves ~1us but races the runtime's end-of-NEFF semaphore
        # reset; an NRT_EXEC_UNIT_UNRECOVERABLE was observed under repeated
        # executions without this wait, so keep it.)
        nc.sync.wait_ge(c, 4)
        nc.sync.dma_start(
            bass.AP(o, 0, [[1, 4]]),
            bass.AP(res_sb, 0, [[4, 1], [1, 4]]),
        ).then_inc(semO, 16)
        nc.sync.wait_ge(semO, 16)
    return nc


def _get_nc():
    global _NC_CACHE
    if _NC_CACHE is None:
        _NC_CACHE = _build_nc()
    return _NC_CACHE


def _run(x, matrices, **kwargs):
    """Uncached path (kept for test harnesses that want BassKernelResults)."""
    nc = _get_nc()
    in_map = {
        "x": np.ascontiguousarray(x, dtype=np.float32),
        "matrices": np.ascontiguousarray(matrices, dtype=np.float32),
    }
    in_maps = [in_map for _ in range(N_CORES)]
    return run_bass_kernel_spmd(nc, in_maps, list(range(N_CORES)), **kwargs)


def kernel(x, matrices):
    # Fresh dispatch per call (the ecosystem-default run_bass_kernel_spmd
    # path). Each call executes the NEFF as a first execution, which has a
    # ~8us faster runtime prologue than re-executing a cached executable
    # (re-execution repeats the engine-state TENSOR_LOAD round). The
    # compiled NEFF itself comes from the on-disk neuron compile cache, so
    # per-call overhead is only the PJRT trace+load (~0.7s wall).
    res = _run(x, matrices)
    return np.asarray(res.results[0]["out"], dtype=np.float32).reshape(4)
